# revision 13
# baseline (speedup 1.0000x reference)
"""AdaptMultiheadAttention on 8 TRN2 NeuronCores.

Sharding: tensor-parallel over heads (2 heads/core). Per core:
  - qT,kT [128ch, 4096t] via transposed QKV matmul (lhsT=W_qkv cols, rhs=xT)
  - v [4096t, 128ch] via direct matmul (lhsT=xT cols, rhs=W_v)
  - scores^T per (b,h): [s,t] tiles; exp (no max-sub: |scores|<~6); attn@V with
    lhsT=[v|ones] giving outT [c,t] + colsum row in one pass
  - adaptive weight: pos = qT . k_mean via M=1 matmul -> AllReduce(16KB) ->
    min/max/scale rows on partition 64; s_row = aw/colsum; K=1 ones-matmul
    broadcasts s_row across partitions; one DVE mult normalizes outT
  - AllToAll swaps head-blocks for t-blocks; proj consumes received [c,t]
    blocks directly as lhsT against full W_proj; output rows are disjoint.
Compute dtype bf16 (f32 PSUM accumulation), f32 softmax/aw arithmetic.
Measured end-to-end rel err vs f32 reference ~5.5e-3.
"""
import sys

if '/opt/trn_rl_repo' not in sys.path:
    sys.path.insert(0, '/opt/trn_rl_repo')

import math
import os
import numpy as np
import ml_dtypes

import concourse.bacc as bacc
import concourse.mybir as mybir
import concourse.tile as tile
from concourse.bass_utils import run_bass_kernel_spmd

bf16 = ml_dtypes.bfloat16
F32 = mybir.dt.float32
BF16 = mybir.dt.bfloat16

B, T, W = 2, 2048, 1024
H, C = 16, 64                  # heads, head dim
NC = 8                         # cores
HL = H // NC                   # heads per core = 2
BT = B * T                     # 4096
SCALE = 1.0 / math.sqrt(math.sqrt(C))
KCH = 8                        # K chunks of 128 over W
NPAN = 4                       # t panels of 512 per (b,)
NST = T // 128                 # s tiles per b = 16
VBLK = 2 * C + 2               # v block cols: [v_h0|1|v_h1|1] = 130

_NC_CACHE = None


def build():
    stage = int(os.environ.get("KSTAGE", "4"))
    nc = bacc.Bacc("TRN2", target_bir_lowering=False, debug=False, num_devices=NC)

    xt_d = nc.declare_dram_parameter("xt", [KCH, 128, BT], BF16, isOutput=False)
    wqk_d = nc.declare_dram_parameter("wqk", [KCH, 128, 256], BF16, isOutput=False)
    wv_d = nc.declare_dram_parameter("wv", [KCH, 128, 128], BF16, isOutput=False)
    wp_d = nc.declare_dram_parameter("wp", [KCH, 128, W], BF16, isOutput=False)
    out_d = nc.declare_dram_parameter("out", [BT // NC, W], F32, isOutput=True)

    pos_in = pos_ar = a2a_in = a2a_out = None
    if stage >= 2:
        pos_in = nc.dram_tensor("pos_in", [1, BT], F32)
        pos_ar = nc.dram_tensor("pos_ar", [1, BT], F32, addr_space="Shared")
    if stage >= 4:
        a2a_in = nc.dram_tensor("a2a_in", [NC, 128, 512], BF16)
        a2a_out = nc.dram_tensor("a2a_out", [NC, 128, 512], BF16)

    with tile.TileContext(nc) as tc:
        with (
            tc.tile_pool(name="w", bufs=1) as pw,
            tc.tile_pool(name="qv", bufs=1) as pqv,
            tc.tile_pool(name="outp", bufs=1) as pout,
        ):
            # ---- resident tiles ----
            wqk_sb = pw.tile([128, KCH * 256], BF16, tag="wqk")
            nc.sync.dma_start(
                wqk_sb[:, :], wqk_d[:, :, :].rearrange("k p j -> p k j"))
            wv_sb = pw.tile([128, KCH * 128], BF16, tag="wv")
            nc.sync.dma_start(
                wv_sb[:, :], wv_d[:, :, :].rearrange("k p j -> p k j"))
            wp_sb = pw.tile([128, KCH * W], BF16, tag="wp")
            nc.sync.dma_start(
                wp_sb[:, :], wp_d[:, :, :].rearrange("k p j -> p k j"))

            qT = pqv.tile([128, BT], BF16, tag="qT")     # scaled
            kT = pqv.tile([128, BT], BF16, tag="kT")     # scaled
            v_sb = pqv.tile([128, NST * B * VBLK], BF16, tag="v")
            scr = pw.tile([128, BT], F32, tag="scr")     # row0: pos, row64: pos_ar->aw
            mnr = pw.tile([128, 8], F32, tag="mnr")      # p64: mn[b], r[b]
            km = pw.tile([128, 2], BF16, tag="km")
            km_f = pw.tile([128, 2], F32, tag="km_f")
            ones = pw.tile([128, 64], BF16, tag="ones")
            rc = pw.tile([128, 512], F32, tag="rc")
            srow = pw.tile([128, 512], BF16, tag="srow")
            outT = [pout.tile([64, BT], BF16, tag=f"outT{h}", name=f"outT{h}")
                    for h in range(HL)]

            nc.vector.memset(ones[64:65, :], 1.0)
            # ones columns of v blocks
            v_view = v_sb.rearrange("p (s c) -> p s c", c=VBLK)
            nc.vector.memset(v_view[:, :, C:C + 1], 1.0)
            nc.vector.memset(v_view[:, :, 2 * C + 1:2 * C + 2], 1.0)

            # ---- phase 1: QKV ----
            with (
                tc.tile_pool(name="xt", bufs=1) as pxt,
                tc.tile_pool(name="ps1", bufs=2, space="PSUM") as ps1,
            ):
                xt = []
                for k in range(KCH):
                    t_ = pxt.tile([128, BT], BF16, tag=f"xt{k}")
                    nc.sync.dma_start(t_[:, :], xt_d[k])
                    xt.append(t_)

                # qT, kT (scaled on PSUM->SBUF copy)
                for m, dst in (((0, qT), (1, kT)) if stage >= 1 else ()):
                    for nb in range(BT // 512):
                        ps = ps1.tile([128, 512], F32, tag="qk")
                        for k in range(KCH):
                            nc.tensor.matmul(
                                ps[:, :],
                                wqk_sb[:, k * 256 + m * 128: k * 256 + (m + 1) * 128],
                                xt[k][:, nb * 512:(nb + 1) * 512],
                                start=(k == 0), stop=(k == KCH - 1))
                        nc.scalar.activation(
                            dst[:, nb * 512:(nb + 1) * 512], ps[:, :],
                            mybir.ActivationFunctionType.Copy, scale=SCALE)

                # v rows [t, 128] -> packed v blocks
                for tb in range(BT // 128 if stage >= 1 else 0):
                    ps = ps1.tile([128, 128], F32, tag="v")
                    for k in range(KCH):
                        nc.tensor.matmul(
                            ps[:, :],
                            xt[k][:, tb * 128:(tb + 1) * 128],
                            wv_sb[:, k * 128:(k + 1) * 128],
                            start=(k == 0), stop=(k == KCH - 1))
                    base = tb * VBLK
                    nc.scalar.activation(
                        v_sb[:, base:base + C], ps[:, 0:C],
                        mybir.ActivationFunctionType.Copy)
                    nc.scalar.activation(
                        v_sb[:, base + C + 1:base + 2 * C + 1], ps[:, C:2 * C],
                        mybir.ActivationFunctionType.Copy)

                # k_mean (scaled) and pos partials
                for b in range(B if stage >= 1 else 0):
                    nc.vector.tensor_reduce(
                        km_f[:, b:b + 1], kT[:, b * T:(b + 1) * T],
                        axis=mybir.AxisListType.X, op=mybir.AluOpType.add)
                if stage >= 1:
                    nc.scalar.activation(km[:, :], km_f[:, :],
                                         mybir.ActivationFunctionType.Copy,
                                         scale=1.0 / T)
                for nb in range(BT // 512 if stage >= 1 else 0):
                    b = nb // NPAN
                    ps = ps1.tile([128, 512], F32, tag="qk")
                    nc.tensor.matmul(ps[0:1, :], km[:, b:b + 1],
                                     qT[:, nb * 512:(nb + 1) * 512],
                                     start=True, stop=True)
                    nc.vector.tensor_copy(scr[0:1, nb * 512:(nb + 1) * 512],
                                          ps[0:1, :])

            # pos AllReduce (16KB) + aw rows on partition 64
            if stage >= 2:
                nc.sync.dma_start(pos_in[:, :], scr[0:1, :])
                nc.gpsimd.collective_compute(
                    "AllReduce", mybir.AluOpType.add,
                    replica_groups=[list(range(NC))],
                    ins=[pos_in.ap().opt()], outs=[pos_ar.ap().opt()])
                nc.sync.dma_start(scr[64:65, :], pos_ar[:, :])
            for b in range(B if stage >= 2 else 0):
                sl = scr[64:65, b * T:(b + 1) * T]
                nc.vector.tensor_reduce(mnr[64:65, b:b + 1], sl,
                                        axis=mybir.AxisListType.X,
                                        op=mybir.AluOpType.min)
                nc.vector.tensor_reduce(mnr[64:65, 2 + b:3 + b], sl,
                                        axis=mybir.AxisListType.X,
                                        op=mybir.AluOpType.max)
                nc.vector.tensor_sub(mnr[64:65, 4 + b:5 + b],
                                     mnr[64:65, 2 + b:3 + b],
                                     mnr[64:65, b:b + 1])
                nc.vector.tensor_scalar_add(mnr[64:65, 4 + b:5 + b],
                                            mnr[64:65, 4 + b:5 + b], 1e-6)
                nc.vector.reciprocal(mnr[64:65, 6 + b:7 + b],
                                     mnr[64:65, 4 + b:5 + b])
                # aw in place over pos_ar row: (pos-mn)*r
                nc.vector.tensor_scalar(sl, sl,
                                        scalar1=mnr[64:65, b:b + 1],
                                        scalar2=mnr[64:65, 6 + b:7 + b],
                                        op0=mybir.AluOpType.subtract,
                                        op1=mybir.AluOpType.mult)

            # ---- phase 2: attention ----
            with (
                tc.tile_pool(name="exp", bufs=2) as pexp,
                tc.tile_pool(name="ps2", bufs=3, space="PSUM") as ps2,
                tc.tile_pool(name="ps2b", bufs=2, space="PSUM") as ps2b,
            ):
                for b in range(B if stage >= 3 else 0):
                    for hl in range(HL):
                        hb = hl * C
                        for p in range(NPAN):
                            t0 = b * T + p * 512
                            exps = []
                            for si in range(NST):
                                s0 = b * T + si * 128
                                ps = ps2.tile([128, 512], F32, tag="st")
                                nc.tensor.matmul(
                                    ps[:, :],
                                    kT[hb:hb + C, s0:s0 + 128],
                                    qT[hb:hb + C, t0:t0 + 512],
                                    start=True, stop=True)
                                ex = pexp.tile([128, 512], BF16, tag=f"e{si}")
                                nc.scalar.activation(
                                    ex[:, :], ps[:, :],
                                    mybir.ActivationFunctionType.Exp)
                                exps.append(ex)
                            po = ps2b.tile([128, 512], F32, tag="po")
                            for si in range(NST):
                                vb = (b * NST + si) * VBLK + hl * (C + 1)
                                nc.tensor.matmul(
                                    po[0:C + 1, :],
                                    v_sb[:, vb:vb + C + 1],
                                    exps[si][:, :],
                                    start=(si == 0), stop=(si == NST - 1))
                            nc.vector.reciprocal(rc[64:65, :], po[C:C + 1, :])
                            nc.vector.tensor_mul(srow[64:65, :], rc[64:65, :],
                                                 scr[64:65, t0:t0 + 512])
                            bc = ps2b.tile([128, 512], F32, tag="bc")
                            nc.tensor.matmul(bc[0:C, :], ones[64:65, :],
                                             srow[64:65, :],
                                             start=True, stop=True)
                            bcs = pexp.tile([128, 512], F32, tag="bcs",
                                            name="bcs")
                            nc.vector.tensor_copy(bcs[0:C, :], bc[0:C, :])
                            nc.vector.tensor_mul(
                                outT[hl][0:C, t0:t0 + 512],
                                po[0:C, :], bcs[0:C, :])

            # ---- phase 3: AllToAll + proj ----
            for j in (range(NC) if stage >= 4 else []):
                nc.sync.dma_start(a2a_in[j, 0:64, :],
                                  outT[0][:, j * 512:(j + 1) * 512])
                nc.sync.dma_start(a2a_in[j, 64:128, :],
                                  outT[1][:, j * 512:(j + 1) * 512])
            if stage >= 4:
                nc.gpsimd.collective_compute(
                    "AllToAll", mybir.AluOpType.bypass,
                    replica_groups=[list(range(NC))],
                    ins=[a2a_in.ap().opt()], outs=[a2a_out.ap().opt()])

            if stage >= 4:
                with (
                    tc.tile_pool(name="ag", bufs=1) as pag,
                    tc.tile_pool(name="ps3", bufs=4, space="PSUM") as ps3,
                ):
                    ag = pag.tile([128, NC * 512], BF16, tag="ag")
                    nc.sync.dma_start(
                        ag[:, :], a2a_out[:, :, :].rearrange("g p t -> p g t"))
                    for tb in range(4):
                        of = pw.tile([128, W], F32, tag="of")
                        for nh in range(2):
                            ps = ps3.tile([128, 512], F32, tag="f")
                            for g in range(NC):
                                nc.tensor.matmul(
                                    ps[:, :],
                                    ag[:, g * 512 + tb * 128: g * 512 + (tb + 1) * 128],
                                    wp_sb[:, g * W + nh * 512: g * W + (nh + 1) * 512],
                                    start=(g == 0), stop=(g == NC - 1))
                            nc.scalar.activation(
                                of[:, nh * 512:(nh + 1) * 512], ps[:, :],
                                mybir.ActivationFunctionType.Copy)
                        nc.sync.dma_start(out_d[tb * 128:(tb + 1) * 128, :],
                                          of[:, :])

    nc.compile()
    return nc


def _prep_inputs(x, W_qkv, W_proj):
    xt = np.ascontiguousarray(
        x.reshape(BT, W).T.astype(bf16)).reshape(KCH, 128, BT)
    wp = np.ascontiguousarray(W_proj.astype(bf16)).reshape(KCH, 128, W)
    in_maps = []
    for c in range(NC):
        h0, h1 = 2 * c, 2 * c + 1
        cols_qk = np.concatenate([
            np.arange(h0 * 192, h0 * 192 + 64),
            np.arange(h1 * 192, h1 * 192 + 64),
            np.arange(h0 * 192 + 64, h0 * 192 + 128),
            np.arange(h1 * 192 + 64, h1 * 192 + 128)])
        cols_v = np.concatenate([
            np.arange(h0 * 192 + 128, h0 * 192 + 192),
            np.arange(h1 * 192 + 128, h1 * 192 + 192)])
        wqk = np.ascontiguousarray(
            W_qkv[:, cols_qk].astype(bf16)).reshape(KCH, 128, 256)
        wv = np.ascontiguousarray(
            W_qkv[:, cols_v].astype(bf16)).reshape(KCH, 128, 128)
        in_maps.append({"xt": xt, "wqk": wqk, "wv": wv, "wp": wp})
    return in_maps


def run(inputs, trace=False):
    global _NC_CACHE
    if _NC_CACHE is None:
        _NC_CACHE = build()
    nc = _NC_CACHE
    x = np.asarray(inputs["x"], dtype=np.float32)
    W_qkv = np.asarray(inputs["W_qkv"], dtype=np.float32)
    W_proj = np.asarray(inputs["W_proj"], dtype=np.float32)
    in_maps = _prep_inputs(x, W_qkv, W_proj)
    res = run_bass_kernel_spmd(nc, in_maps, core_ids=list(range(NC)), trace=trace)
    out = np.concatenate([res.results[c]["out"] for c in range(NC)], axis=0)
    return out.reshape(B, T, W).astype(np.float32), res.exec_time_ns


def kernel(**inputs):
    out, _ = run(inputs)
    return out


# revision 17
# speedup vs baseline: 1.0602x; 1.0602x over previous
"""AdaptMultiheadAttention on 8 TRN2 NeuronCores.

Sharding: tensor-parallel over heads (2 heads/core). Per core:
  - qT,kT [128ch, 4096t] via transposed QKV matmul (lhsT=W_qkv cols, rhs=xT)
  - v [4096t, 128ch] via direct matmul (lhsT=xT cols, rhs=W_v)
  - scores^T per (b,h): [s,t] tiles; exp (no max-sub: |scores|<~6); attn@V with
    lhsT=[v|ones] giving outT [c,t] + colsum row in one pass
  - adaptive weight: pos = qT . k_mean via M=1 matmul -> AllReduce(16KB) ->
    min/max/scale rows on partition 64; s_row = aw/colsum; K=1 ones-matmul
    broadcasts s_row across partitions; one DVE mult normalizes outT
  - AllToAll swaps head-blocks for t-blocks; proj consumes received [c,t]
    blocks directly as lhsT against full W_proj; output rows are disjoint.
Compute dtype bf16 (f32 PSUM accumulation), f32 softmax/aw arithmetic.
Measured end-to-end rel err vs f32 reference ~5.5e-3.
"""
import sys

if '/opt/trn_rl_repo' not in sys.path:
    sys.path.insert(0, '/opt/trn_rl_repo')

import math
import os
import numpy as np
import ml_dtypes

import concourse.bacc as bacc
import concourse.mybir as mybir
import concourse.tile as tile
from concourse.bass_utils import run_bass_kernel_spmd

bf16 = ml_dtypes.bfloat16
F32 = mybir.dt.float32
BF16 = mybir.dt.bfloat16

B, T, W = 2, 2048, 1024
H, C = 16, 64                  # heads, head dim
NC = 8                         # cores
HL = H // NC                   # heads per core = 2
BT = B * T                     # 4096
SCALE = 1.0 / math.sqrt(math.sqrt(C))
KCH = 8                        # K chunks of 128 over W
NPAN = 4                       # t panels of 512 per (b,)
NST = T // 128                 # s tiles per b = 16
VBLK = 2 * C + 2               # v block cols: [v_h0|1|v_h1|1] = 130

_NC_CACHE = None


def build():
    stage = int(os.environ.get("KSTAGE", "4"))
    nc = bacc.Bacc("TRN2", target_bir_lowering=False, debug=False, num_devices=NC)

    xt_d = nc.declare_dram_parameter("xt", [KCH, 128, BT], BF16, isOutput=False)
    wqk_d = nc.declare_dram_parameter("wqk", [KCH, 128, 256], BF16, isOutput=False)
    wv_d = nc.declare_dram_parameter("wv", [KCH, 128, 128], BF16, isOutput=False)
    wp_d = nc.declare_dram_parameter("wp", [KCH, 128, W], BF16, isOutput=False)
    out_d = nc.declare_dram_parameter("out", [BT // NC, W], F32, isOutput=True)

    pos_in = pos_ar = a2a_in = a2a_out = None
    if stage >= 2:
        pos_in = nc.dram_tensor("pos_in", [1, BT], F32)
        pos_ar = nc.dram_tensor("pos_ar", [1, BT], F32, addr_space="Shared")
    if stage >= 4:
        a2a_in = nc.dram_tensor("a2a_in", [NC, 128, 512], BF16)
        a2a_out = nc.dram_tensor("a2a_out", [NC, 128, 512], BF16)

    with tile.TileContext(nc) as tc:
        with (
            tc.tile_pool(name="w", bufs=1) as pw,
            tc.tile_pool(name="qv", bufs=1) as pqv,
            tc.tile_pool(name="outp", bufs=1) as pout,
        ):
            # ---- resident tiles ----
            wqk_sb = pw.tile([128, KCH * 256], BF16, tag="wqk")
            nc.sync.dma_start(
                wqk_sb[:, :], wqk_d[:, :, :].rearrange("k p j -> p k j"))
            wv_sb = pw.tile([128, KCH * 128], BF16, tag="wv")
            nc.sync.dma_start(
                wv_sb[:, :], wv_d[:, :, :].rearrange("k p j -> p k j"))
            wp_sb = pw.tile([128, KCH * W], BF16, tag="wp")
            nc.sync.dma_start(
                wp_sb[:, :], wp_d[:, :, :].rearrange("k p j -> p k j"))

            qT = pqv.tile([128, BT], BF16, tag="qT")     # scaled
            kT = pqv.tile([128, BT], BF16, tag="kT")     # scaled
            v_sb = pqv.tile([128, NST * B * VBLK], BF16, tag="v")
            scr = pw.tile([128, BT], F32, tag="scr")     # row0: pos, row64: pos_ar->aw
            mnr = pw.tile([128, 8], F32, tag="mnr")      # p64: mn[b], r[b]
            km = pw.tile([128, 2], BF16, tag="km")
            km_f = pw.tile([128, 2], F32, tag="km_f")
            ones = pw.tile([128, 64], BF16, tag="ones")
            srow = pw.tile([128, 1024], BF16, tag="srow")
            rc = pw.tile([128, 1024], F32, tag="rc")
            outT = [pout.tile([64, BT], BF16, tag=f"outT{h}", name=f"outT{h}")
                    for h in range(HL)]

            nc.vector.memset(ones[64:65, :], 1.0)
            # ones columns of v blocks
            v_view = v_sb.rearrange("p (s c) -> p s c", c=VBLK)
            nc.vector.memset(v_view[:, :, C:C + 1], 1.0)
            nc.vector.memset(v_view[:, :, 2 * C + 1:2 * C + 2], 1.0)

            # ---- phase 1: QKV ----
            with (
                tc.tile_pool(name="xt", bufs=1) as pxt,
                tc.tile_pool(name="ps1", bufs=2, space="PSUM") as ps1,
            ):
                xt = []
                for k in range(KCH):
                    t_ = pxt.tile([128, BT], BF16, tag=f"xt{k}")
                    nc.sync.dma_start(t_[:, :], xt_d[k])
                    xt.append(t_)

                # qT, kT (scaled on PSUM->SBUF copy)
                for m, dst in (((0, qT), (1, kT)) if stage >= 1 else ()):
                    for nb in range(BT // 512):
                        ps = ps1.tile([128, 512], F32, tag="qk")
                        for k in range(KCH):
                            nc.tensor.matmul(
                                ps[:, :],
                                wqk_sb[:, k * 256 + m * 128: k * 256 + (m + 1) * 128],
                                xt[k][:, nb * 512:(nb + 1) * 512],
                                start=(k == 0), stop=(k == KCH - 1))
                        nc.vector.tensor_scalar_mul(
                            dst[:, nb * 512:(nb + 1) * 512], ps[:, :], SCALE)

                # v rows [t, 128] -> packed v blocks
                for tb in range(BT // 128 if stage >= 1 else 0):
                    ps = ps1.tile([128, 128], F32, tag="v")
                    for k in range(KCH):
                        nc.tensor.matmul(
                            ps[:, :],
                            xt[k][:, tb * 128:(tb + 1) * 128],
                            wv_sb[:, k * 128:(k + 1) * 128],
                            start=(k == 0), stop=(k == KCH - 1))
                    base = tb * VBLK
                    nc.vector.tensor_copy(v_sb[:, base:base + C], ps[:, 0:C])
                    nc.vector.tensor_copy(
                        v_sb[:, base + C + 1:base + 2 * C + 1], ps[:, C:2 * C])

                # k_mean (scaled) and pos partials
                for b in range(B if stage >= 1 else 0):
                    nc.vector.tensor_reduce(
                        km_f[:, b:b + 1], kT[:, b * T:(b + 1) * T],
                        axis=mybir.AxisListType.X, op=mybir.AluOpType.add)
                if stage >= 1:
                    nc.vector.tensor_scalar_mul(km[:, :], km_f[:, :], 1.0 / T)
                for nb in range(BT // 512 if stage >= 1 else 0):
                    b = nb // NPAN
                    ps = ps1.tile([128, 512], F32, tag="qk")
                    nc.tensor.matmul(ps[0:1, :], km[:, b:b + 1],
                                     qT[:, nb * 512:(nb + 1) * 512],
                                     start=True, stop=True)
                    nc.vector.tensor_copy(scr[0:1, nb * 512:(nb + 1) * 512],
                                          ps[0:1, :])

            # pos AllReduce (16KB) + aw rows on partition 64
            if stage >= 2:
                nc.sync.dma_start(pos_in[:, :], scr[0:1, :])
                nc.gpsimd.collective_compute(
                    "AllReduce", mybir.AluOpType.add,
                    replica_groups=[list(range(NC))],
                    ins=[pos_in.ap().opt()], outs=[pos_ar.ap().opt()])
                nc.sync.dma_start(scr[64:65, :], pos_ar[:, :])
            for b in range(B if stage >= 2 else 0):
                sl = scr[64:65, b * T:(b + 1) * T]
                nc.vector.tensor_reduce(mnr[64:65, b:b + 1], sl,
                                        axis=mybir.AxisListType.X,
                                        op=mybir.AluOpType.min)
                nc.vector.tensor_reduce(mnr[64:65, 2 + b:3 + b], sl,
                                        axis=mybir.AxisListType.X,
                                        op=mybir.AluOpType.max)
                nc.vector.tensor_sub(mnr[64:65, 4 + b:5 + b],
                                     mnr[64:65, 2 + b:3 + b],
                                     mnr[64:65, b:b + 1])
                nc.vector.tensor_scalar_add(mnr[64:65, 4 + b:5 + b],
                                            mnr[64:65, 4 + b:5 + b], 1e-6)
                nc.vector.reciprocal(mnr[64:65, 6 + b:7 + b],
                                     mnr[64:65, 4 + b:5 + b])
                # aw in place over pos_ar row: (pos-mn)*r
                nc.vector.tensor_scalar(sl, sl,
                                        scalar1=mnr[64:65, b:b + 1],
                                        scalar2=mnr[64:65, 6 + b:7 + b],
                                        op0=mybir.AluOpType.subtract,
                                        op1=mybir.AluOpType.mult)

            # ---- phase 2: attention ----
            # 1024-wide t panels, software-pipelined: unit u+1's score MMs
            # interleave with unit u's attn@V MMs so PE never waits on exp.
            PAN = 1024
            with (
                tc.tile_pool(name="exp", bufs=2) as pexp,
                tc.tile_pool(name="ps2", bufs=2, space="PSUM") as ps2,
                tc.tile_pool(name="ps2b", bufs=2, space="PSUM") as ps2b,
            ):
                units = [(b, hl, p) for b in range(B if stage >= 3 else 0)
                         for hl in range(HL) for p in range(T // PAN)]

                def emit_tail(u, po):
                    b, hl, p = u
                    t0 = b * T + p * PAN
                    nc.vector.reciprocal(rc[64:65, 0:PAN], po[C:C + 1, :])
                    nc.vector.tensor_mul(srow[64:65, 0:PAN], rc[64:65, 0:PAN],
                                         scr[64:65, t0:t0 + PAN])
                    bc = ps2.tile([128, PAN], F32, tag="st", name="bc")
                    for hf in range(2):
                        nc.tensor.matmul(
                            bc[0:C, hf * 512:(hf + 1) * 512], ones[64:65, 0:C],
                            srow[64:65, hf * 512:(hf + 1) * 512],
                            start=True, stop=True)
                    bcs = pexp.tile([128, PAN], F32, tag="bcs", name="bcs")
                    nc.vector.tensor_copy(bcs[0:C, :], bc[0:C, :])
                    nc.vector.tensor_mul(
                        outT[hl][0:C, t0:t0 + PAN], po[0:C, :], bcs[0:C, :])

                prev = None      # (unit, po_tile, exps)
                for u in units:
                    b, hl, p = u
                    hb = hl * C
                    t0 = b * T + p * PAN
                    po = ps2b.tile([128, PAN], F32, tag="po", name="po")
                    exps = []
                    for si in range(NST):
                        s0 = b * T + si * 128
                        ps = ps2.tile([128, PAN], F32, tag="st", name="st")
                        for hf in range(2):
                            nc.tensor.matmul(
                                ps[:, hf * 512:(hf + 1) * 512],
                                kT[hb:hb + C, s0:s0 + 128],
                                qT[hb:hb + C, t0 + hf * 512:t0 + (hf + 1) * 512],
                                start=True, stop=True)
                        ex = pexp.tile([128, PAN], BF16, tag=f"e{si}",
                                       name=f"e{si}")
                        nc.scalar.activation(ex[:, :], ps[:, :],
                                             mybir.ActivationFunctionType.Exp)
                        exps.append(ex)
                        if prev is not None:
                            ub, po_p, exps_p = prev
                            vb = (ub[0] * NST + si) * VBLK + ub[1] * (C + 1)
                            for hf in range(2):
                                nc.tensor.matmul(
                                    po_p[0:C + 1, hf * 512:(hf + 1) * 512],
                                    v_sb[:, vb:vb + C + 1],
                                    exps_p[si][:, hf * 512:(hf + 1) * 512],
                                    start=(si == 0), stop=(si == NST - 1))
                    if prev is not None:
                        emit_tail(prev[0], prev[1])
                    prev = (u, po, exps)
                if prev is not None:
                    ub, po_p, exps_p = prev
                    for si in range(NST):
                        vb = (ub[0] * NST + si) * VBLK + ub[1] * (C + 1)
                        for hf in range(2):
                            nc.tensor.matmul(
                                po_p[0:C + 1, hf * 512:(hf + 1) * 512],
                                v_sb[:, vb:vb + C + 1],
                                exps_p[si][:, hf * 512:(hf + 1) * 512],
                                start=(si == 0), stop=(si == NST - 1))
                    emit_tail(ub, po_p)

            # ---- phase 3: AllToAll + proj ----
            for j in (range(NC) if stage >= 4 else []):
                nc.sync.dma_start(a2a_in[j, 0:64, :],
                                  outT[0][:, j * 512:(j + 1) * 512])
                nc.sync.dma_start(a2a_in[j, 64:128, :],
                                  outT[1][:, j * 512:(j + 1) * 512])
            if stage >= 4:
                nc.gpsimd.collective_compute(
                    "AllToAll", mybir.AluOpType.bypass,
                    replica_groups=[list(range(NC))],
                    ins=[a2a_in.ap().opt()], outs=[a2a_out.ap().opt()])

            if stage >= 4:
                with (
                    tc.tile_pool(name="ag", bufs=1) as pag,
                    tc.tile_pool(name="ps3", bufs=4, space="PSUM") as ps3,
                ):
                    ag = pag.tile([128, NC * 512], BF16, tag="ag")
                    nc.sync.dma_start(
                        ag[:, :], a2a_out[:, :, :].rearrange("g p t -> p g t"))
                    for tb in range(4):
                        of = pw.tile([128, W], F32, tag="of")
                        for nh in range(2):
                            ps = ps3.tile([128, 512], F32, tag="f")
                            for g in range(NC):
                                nc.tensor.matmul(
                                    ps[:, :],
                                    ag[:, g * 512 + tb * 128: g * 512 + (tb + 1) * 128],
                                    wp_sb[:, g * W + nh * 512: g * W + (nh + 1) * 512],
                                    start=(g == 0), stop=(g == NC - 1))
                            nc.scalar.activation(
                                of[:, nh * 512:(nh + 1) * 512], ps[:, :],
                                mybir.ActivationFunctionType.Copy)
                        nc.sync.dma_start(out_d[tb * 128:(tb + 1) * 128, :],
                                          of[:, :])

    nc.compile()
    return nc


def _prep_inputs(x, W_qkv, W_proj):
    xt = np.ascontiguousarray(
        x.reshape(BT, W).T.astype(bf16)).reshape(KCH, 128, BT)
    wp = np.ascontiguousarray(W_proj.astype(bf16)).reshape(KCH, 128, W)
    in_maps = []
    for c in range(NC):
        h0, h1 = 2 * c, 2 * c + 1
        cols_qk = np.concatenate([
            np.arange(h0 * 192, h0 * 192 + 64),
            np.arange(h1 * 192, h1 * 192 + 64),
            np.arange(h0 * 192 + 64, h0 * 192 + 128),
            np.arange(h1 * 192 + 64, h1 * 192 + 128)])
        cols_v = np.concatenate([
            np.arange(h0 * 192 + 128, h0 * 192 + 192),
            np.arange(h1 * 192 + 128, h1 * 192 + 192)])
        wqk = np.ascontiguousarray(
            W_qkv[:, cols_qk].astype(bf16)).reshape(KCH, 128, 256)
        wv = np.ascontiguousarray(
            W_qkv[:, cols_v].astype(bf16)).reshape(KCH, 128, 128)
        in_maps.append({"xt": xt, "wqk": wqk, "wv": wv, "wp": wp})
    return in_maps


def run(inputs, trace=False):
    global _NC_CACHE
    if _NC_CACHE is None:
        _NC_CACHE = build()
    nc = _NC_CACHE
    x = np.asarray(inputs["x"], dtype=np.float32)
    W_qkv = np.asarray(inputs["W_qkv"], dtype=np.float32)
    W_proj = np.asarray(inputs["W_proj"], dtype=np.float32)
    in_maps = _prep_inputs(x, W_qkv, W_proj)
    res = run_bass_kernel_spmd(nc, in_maps, core_ids=list(range(NC)), trace=trace)
    out = np.concatenate([res.results[c]["out"] for c in range(NC)], axis=0)
    return out.reshape(B, T, W).astype(np.float32), res.exec_time_ns


def kernel(**inputs):
    out, _ = run(inputs)
    return out
